# revision 6
# baseline (speedup 1.0000x reference)
"""Causal attention head (B=4, T=4096, D=1024, H=64) on 8 TRN2 NeuronCores.

Sharding: 2 cores per batch element. Within a batch, core role r in {0,1}
owns the interleaved query rows {256*v + 2*i + r : v in [0,16), i in [0,128)}.
Every core runs an IDENTICAL instruction stream (SPMD-uniform). The role
offset r lives entirely in host-staged data: the per-core x^T has its
columns pair-swapped for r=1 (token c^r at column c), so the core's own
query rows sit at even columns {256*v + 2*i} for both roles and the causal
masks (which absorb the within-pair order) are per-core input data.

Per-core device program:
  - all small constants (Wq | Wk|Wv | identity | masks) in ONE fat blob DMA
    on the scalar HWDGE queue; x^T [D,T] bf16 streamed in 512-column slices
    on the sync HWDGE queue
  - PE warmup matmuls on scratch during the initial DMA wait (keeps HAM at
    full clock)
  - K^T/V^T projection with lhsT=[Wk|Wv]; Q^T projection reads own-row
    columns straight out of x^T with a stride-2 access pattern
  - V^T -> V via PE transposes; V tiles stored as [128,65] with a ones
    column (fused softmax denominator)
  - flash-style attention over four 512-query-column groups: per group g,
    S^T strips [128 x <=512] in PSUM for key tiles j=0..8g+7 (trimmed at
    the causal diagonal), exp on ScalarE (scale=1/8) -> P^T bf16, causal
    mask multiply on GpSimd for diagonal tiles, ctx^T[65,512] accumulated
    in PSUM per group; group epilogue (copy + DMA out) overlaps the next
    group's strips
Host side: shard/cast/permute inputs, gather + re-interleave outputs.
"""

import numpy as np
import ml_dtypes

import concourse.tile as tile
import concourse.mybir as mybir
from concourse import bacc
from concourse.bass_utils import run_bass_kernel_spmd

BF16 = ml_dtypes.bfloat16
F32 = np.float32

B, T, D, H = 4, 4096, 1024, 64
TL = 2048          # local query columns per core
N_CORES = 8
NKT = T // 128     # 32 key tiles
NV = TL // 128     # 16 virtual query tiles
DCH = D // 128     # 8 contraction chunks
DT_BF = mybir.dt.bfloat16
DT_F32 = mybir.dt.float32
EXP = mybir.ActivationFunctionType.Exp
MUL = mybir.AluOpType.mult

N_WARM = 130        # scratch matmuls to warm the PE during the DMA wait

# constant-blob column layout (bf16, 128 partitions)
CB_WQ = 0                  # [d, 64]  -> 512 cols
CB_WKV = 512               # [d, 128] -> 1024 cols
CB_ID = 1536               # [128, 64] identity (bottom half)
CB_MASK = 1600             # [128, 256] diagonal masks A|B
CB_COLS = 1856


def _build():
    nc = bacc.Bacc("TRN2", target_bir_lowering=False, debug=False,
                   num_devices=N_CORES)

    xt = nc.dram_tensor("xt", [128, DCH * T], DT_BF, kind="ExternalInput").ap()
    cb = nc.dram_tensor("cb", [128, CB_COLS], DT_BF, kind="ExternalInput").ap()
    y = nc.dram_tensor("y", [65, TL], DT_F32, kind="ExternalOutput").ap()

    with tile.TileContext(nc) as tc:
        _body(nc, tc, xt, cb, y)

    nc.compile()
    return nc


def _body(nc, tc, xt, cb, y):
    from contextlib import ExitStack

    es = ExitStack()
    with es:
        pp = es.enter_context(tc.tile_pool(name="persist", bufs=1))
        xt_sb = pp.tile([128, DCH * T], DT_BF)
        cb_sb = pp.tile([128, CB_COLS], DT_BF)
        kvT_sb = pp.tile([128, T], DT_BF)       # rows 0:64 = K^T, 64:128 = V^T
        kvT2_sb = pp.tile([128, T], DT_BF)      # rows 64:128 = K^T copy
        qT_sb = pp.tile([64, TL], DT_BF)
        qT2_sb = pp.tile([128, TL], DT_BF)      # rows 64:128 = Q^T copy
        vones_sb = pp.tile([128, NKT * 65], DT_BF)  # V tiles + ones col
        warm_sb = pp.tile([128, 64], DT_BF)

        # ---- input DMAs ----
        # one fat constant blob on the scalar HWDGE queue (parallel to sync)
        nc.scalar.dma_start(cb_sb[:], cb[:])
        # x^T stream on the sync HWDGE queue. SBUF layout is SLICE-major
        # [p, s(8), d(8), c(512)] so each slice DMA writes one contiguous
        # 4096-column block and downstream reads depend on exactly the
        # slices they touch (no false interval-overlap deps).
        for s in range(8):
            nc.sync.dma_start(xt_sb[:, s * 4096:(s + 1) * 4096],
                              xt[:, s * 4096:(s + 1) * 4096])

        nc.gpsimd.memset(warm_sb[:], 0.0)
        nc.vector.memset(vones_sb[:], 1.0)

        # strided view for Q: own query rows at even columns
        # [p, s(8), d(8), vh(2), i(128), par(2)]; par=0 selects own rows
        xt_q = xt_sb.rearrange("p (s d vh i q) -> p s d vh i q",
                               s=8, d=DCH, vh=2, i=128, q=2)

        psum_proj = es.enter_context(
            tc.tile_pool(name="psum_proj", bufs=2, space="PSUM"))
        psum_vt = es.enter_context(
            tc.tile_pool(name="psum_vt", bufs=1, space="PSUM"))

        proj_tiles = {}   # live PSUM tile per in-flight projection

        def emit_q_half(s, h):
            """Half h of q-slice s projection (d chunks 4h..4h+3)."""
            if h == 0:
                proj_tiles["q", s] = psum_proj.tile(
                    [64, 512], DT_F32, name=f"pq{s}", tag="proj")
            pq = proj_tiles["q", s]
            for d in range(4 * h, 4 * h + 4):
                nc.tensor.matmul(
                    pq[:],
                    lhsT=cb_sb[:, CB_WQ + d * H: CB_WQ + (d + 1) * H],
                    rhs=xt_q[:, 2 * s:2 * s + 2, d, :, :, 0:1],
                    start=(d == 0), stop=(d == DCH - 1))
            if h == 1:
                sl = slice(s * 512, (s + 1) * 512)
                nc.vector.tensor_copy(qT_sb[:, sl], pq[:])
                nc.vector.tensor_copy(qT2_sb[64:128, sl], pq[:])
                del proj_tiles["q", s]

        def emit_kv_half(s, h):
            """Half h of kv-slice s projection + V transposes on h==1."""
            if h == 0:
                proj_tiles["kv", s] = psum_proj.tile(
                    [128, 512], DT_F32, name=f"pkv{s}", tag="proj")
            pkv = proj_tiles["kv", s]
            for d in range(4 * h, 4 * h + 4):
                nc.tensor.matmul(
                    pkv[:],
                    lhsT=cb_sb[:, CB_WKV + d * 128: CB_WKV + (d + 1) * 128],
                    rhs=xt_sb[:, s * 4096 + d * 512: s * 4096 + d * 512 + 512],
                    start=(d == 0), stop=(d == DCH - 1))
            if h == 1:
                sl = slice(s * 512, (s + 1) * 512)
                nc.vector.tensor_copy(kvT_sb[:, sl], pkv[:])
                nc.vector.tensor_copy(kvT2_sb[64:128, sl], pkv[0:64, :])
                del proj_tiles["kv", s]
                for t in range(4 * s, 4 * s + 4):
                    pv = psum_vt.tile([128, 64], DT_BF, name=f"pv{t}", tag="pv")
                    nc.tensor.transpose(pv[:],
                                        kvT_sb[64:128, t * 128:(t + 1) * 128],
                                        cb_sb[64:128, CB_ID: CB_ID + 64])
                    nc.vector.tensor_copy(vones_sb[:, t * 65: t * 65 + 64],
                                          pv[:])

        def emit_kv_slice(s):
            emit_kv_half(s, 0)
            emit_kv_half(s, 1)

        def emit_q_slice(s):
            emit_q_half(s, 0)
            emit_q_half(s, 1)

        # ---- PE warmup during DMA wait ----
        wps = psum_proj.tile([64, 64], DT_F32, name="warm", tag="proj")
        for _ in range(N_WARM):
            nc.tensor.matmul(wps[:], lhsT=warm_sb[:, 0:64],
                             rhs=warm_sb[:, 0:64], start=True, stop=True)
        # ramp emissions: kv0 needs only xt slice 0; q0 xt 0-1. Everything
        # else interleaves into the strip loops (g0 pairs 0-1 only touch
        # key tiles 0-3 = kv slice 0).
        emit_kv_slice(0)
        emit_q_slice(0)

        # ---- attention: four 512-query-column groups ----
        # group g covers local q cols [512g, 512g+512), key tiles j=0..8g+7.
        # Later groups' projections are interleaved INTO earlier groups'
        # strip loops (one half-emission per strip pair) so they run in the
        # PE slack while ScalarE paces the exp stream.
        interleave = {
            0: {0: [("kv", 1, 0)], 1: [("kv", 1, 1)],
                2: [("kv", 2, 0)], 3: [("kv", 2, 1), ("q", 1, 0)]},
            1: {0: [("q", 1, 1)], 1: [("kv", 3, 0)], 2: [("kv", 3, 1)],
                3: [("kv", 4, 0)], 4: [("kv", 4, 1)], 5: [("q", 2, 0)],
                6: [("q", 2, 1)], 7: [("kv", 5, 0)]},
            2: {0: [("kv", 5, 1)], 1: [("kv", 6, 0)], 2: [("kv", 6, 1)],
                3: [("q", 3, 0)], 4: [("q", 3, 1)], 5: [("kv", 7, 0)],
                6: [("kv", 7, 1)]},
            3: {},
        }

        with tc.tile_pool(name="psum_ctx", bufs=1, space="PSUM") as pctx, \
             tc.tile_pool(name="psum_strip", bufs=2, space="PSUM") as pstrip, \
             tc.tile_pool(name="pT", bufs=4) as ppT, \
             tc.tile_pool(name="ep_sb", bufs=2) as pes:
            for g in range(4):
                base = 512 * g
                jmax = 8 * g + 7
                ctx_ps = pctx.tile([65, 512], DT_F32, name=f"ctx{g}", tag="ctx")
                pending = []   # deferred ctx matmuls, depth-2 pipeline

                def flush_ctx():
                    fpt, fc_lo, fw, fja, fjb = pending.pop(0)
                    nc.tensor.matmul(
                        ctx_ps[:, fc_lo - base: 512],
                        lhsT=vones_sb[:, fja * 65: fja * 65 + 65],
                        rhs=fpt[:, 512 - fw: 512],
                        start=(fja == 0), stop=False)
                    nc.tensor.matmul(
                        ctx_ps[:, fc_lo - base: 512],
                        lhsT=vones_sb[:, fjb * 65: fjb * 65 + 65],
                        rhs=fpt[:, 512: 512 + fw],
                        start=False, stop=(fjb == jmax))

                # strip pairs: p covers key tiles jA=2p (PE rows 0:63) and
                # jB=2p+1 (PE rows 64:127), concurrent on disjoint row
                # groups. Strip A sits at [512-w,512), B at [512,512+w) so
                # each matmul output stays inside one PSUM bank.
                for p in range(4 * g + 4):
                    for kind, s, h in interleave[g].get(p, ()):
                        (emit_kv_half if kind == "kv" else emit_q_half)(s, h)
                    jA, jB = 2 * p, 2 * p + 1
                    q0 = 128 * p
                    c_lo = max(q0, base)
                    w = base + 512 - c_lo
                    ps = pstrip.tile([128, 1024], DT_F32,
                                     name=f"ps{g}_{p}", tag="ps")
                    nc.tensor.matmul(
                        ps[:, 512 - w: 512],
                        lhsT=kvT_sb[0:64, jA * 128:(jA + 1) * 128],
                        rhs=qT_sb[:, c_lo: base + 512],
                        start=True, stop=True)
                    nc.tensor.matmul(
                        ps[:, 512: 512 + w],
                        lhsT=kvT2_sb[64:128, jB * 128:(jB + 1) * 128],
                        rhs=qT2_sb[64:128, c_lo: base + 512],
                        start=True, stop=True)
                    # ctx matmuls run two pairs behind the scores so the
                    # exp->ctx latency fully hides
                    if len(pending) == 2:
                        flush_ctx()
                    pt = ppT.tile([128, 1024], DT_BF, name=f"pt{g}_{p}", tag="pt")
                    nc.scalar.activation(pt[:, 512 - w: 512 + w],
                                         ps[:, 512 - w: 512 + w],
                                         EXP, bias=0.0, scale=0.125)
                    if c_lo == q0:  # diagonal pair: causal masks, 128 cols each
                        nc.gpsimd.tensor_tensor(
                            pt[:, 512 - w: 512 - w + 128],
                            pt[:, 512 - w: 512 - w + 128],
                            cb_sb[:, CB_MASK: CB_MASK + 128], MUL)
                        nc.gpsimd.tensor_tensor(
                            pt[:, 512: 640], pt[:, 512: 640],
                            cb_sb[:, CB_MASK + 128: CB_MASK + 256], MUL)
                    pending.append((pt, c_lo, w, jA, jB))
                while pending:
                    flush_ctx()
                # group epilogue: raw [num;den]^T out; divide on host
                cs = pes.tile([65, 512], DT_F32, name=f"cs{g}", tag="cs")
                nc.vector.tensor_copy(cs[:], ctx_ps[:])
                nc.sync.dma_start(y[:, base: base + 512], cs[:])


_ROW_IDX = [np.array([256 * v + 2 * i + r for v in range(NV) for i in range(128)])
            for r in range(2)]


def _host_prep(inputs):
    x = np.asarray(inputs["x"], dtype=F32)
    Wk = np.asarray(inputs["Wk"], dtype=F32)
    Wq = np.asarray(inputs["Wq"], dtype=F32)
    Wv = np.asarray(inputs["Wv"], dtype=F32)

    # [d*128+p, t] -> [p, d*t] views for the constant blob
    wq_v = np.ascontiguousarray(
        Wq.reshape(DCH, 128, H).transpose(1, 0, 2).reshape(128, DCH * H))
    wkv = np.concatenate([Wk, Wv], axis=1)
    wkv_v = np.ascontiguousarray(
        wkv.reshape(DCH, 128, 128).transpose(1, 0, 2).reshape(128, DCH * 128))
    identb = np.zeros((128, 64), dtype=F32)
    identb[64:128, :] = np.eye(64, dtype=F32)

    kk = np.arange(128)[:, None]
    ii = np.arange(128)[None, :]
    in_maps = []
    for c in range(N_CORES):
        b, r = c // 2, c % 2
        # pair-swap permutation: token c^r at column c; slice-major
        # device layout [p, s(8), d(8), c(512)]
        perm = np.arange(T) ^ r
        xp = x[b][perm]                      # [T, D]
        xt_np = np.ascontiguousarray(
            xp.reshape(8, 512, DCH, 128).transpose(3, 0, 2, 1)
            .reshape(128, DCH * T)).astype(BF16)
        tok = kk ^ r                      # within-tile token offset at row k
        maskA = (tok <= 2 * ii + r)
        maskB = (tok + 128 <= 2 * ii + r)
        cbn = np.zeros((128, CB_COLS), dtype=F32)
        cbn[:, CB_WQ:CB_WQ + DCH * H] = wq_v
        cbn[:, CB_WKV:CB_WKV + DCH * 128] = wkv_v
        cbn[:, CB_ID:CB_ID + 64] = identb
        cbn[:, CB_MASK:CB_MASK + 128] = maskA
        cbn[:, CB_MASK + 128:CB_MASK + 256] = maskB
        in_maps.append(dict(xt=xt_np, cb=cbn.astype(BF16)))
    return in_maps


def _gather(results):
    out = np.zeros((B, T, H), dtype=F32)
    for c in range(N_CORES):
        b, r = c // 2, c % 2
        yc = results[c]["y"]  # [65, TL]: rows 0:64 = ctx^T, row 64 = denom
        out[b, _ROW_IDX[r]] = (yc[:64, :] / yc[64:65, :]).T
    return out


_NC_CACHE = []


def _execute(inputs, trace=False):
    if not _NC_CACHE:
        _NC_CACHE.append(_build())
    nc = _NC_CACHE[0]
    in_maps = _host_prep(inputs)
    res = run_bass_kernel_spmd(nc, in_maps, core_ids=list(range(N_CORES)),
                               trace=trace)
    return _gather(res.results), res


def kernel(**inputs):
    out, _ = _execute(inputs, trace=False)
    return out

